# revision 16
# baseline (speedup 1.0000x reference)
"""Llama4-style MoE experts (grouped SwiGLU MLP) on Trainium2, 8 NeuronCores.

Expert-parallel: core i runs expert i's full MLP on its 1024-token slice:
    out = (up * silu(gate)) @ W2,  [gate|up] = h @ W1
Per-core shapes: hT [2048, 1024] (host pre-transposed), W1 [2048, 8192],
W2 [4096, 2048].

hidden_states is transposed on the host so h arrives contraction-major;
no on-chip transposes. All matmuls run bf16 on the TensorEngine (f32
operands are cast on the otherwise-idle VectorEngine). Phase B (h @ W1 +
SwiGLU) walks W1 f-blocks [256, 512*7, 256]: the narrow first block
keeps the DMA-bound startup working set small (h + 4 MB) and its
stripes interleave with the h stream per contraction tile, so matmuls
unlock every ~0.75 MB delivered. Phase C (act @ W2) keeps all 8
token-tile PSUM accumulators live and walks the f contraction
innermost, so each W2 tile is consumed in one burst right after it
lands and a small rotating pool gives a deep prefetch; the last 4
f-steps are issued per-token-tile so output drains stagger instead of
bunching at the end.
"""

from contextlib import ExitStack

import numpy as np

import concourse.bass as bass
import concourse.mybir as mybir
import concourse.tile as tile
from concourse import bacc
from concourse.bass_utils import run_bass_kernel_spmd

N_CORES = 8
P = 128
TB = 512  # PSUM free-dim block (one f32 bank)

F32 = mybir.dt.float32
BF16 = mybir.dt.bfloat16
ACT_SIGMOID = mybir.ActivationFunctionType.Sigmoid

# Per-core problem dims (full problem: 8 experts x 1024 tokens, H=2048, F=4096)
T = 1024
H = 2048
F = 4096

# W1 f-block widths (per gate/up half): narrow edges, wide middle
BLOCKS = [(0, 256)] + [(256 + 512 * k, 512) for k in range(7)] + [(3840, 256)]


def build_kernel_body(tc, T=T, H=H, F=F):
    nc = tc.nc
    h_d = nc.dram_tensor("hidden_states", [H, T], F32, kind="ExternalInput").ap()
    w1_d = nc.dram_tensor("gate_up_proj", [H, 2 * F], F32, kind="ExternalInput").ap()
    w2_d = nc.dram_tensor("down_proj", [F, H], F32, kind="ExternalInput").ap()
    out_d = nc.dram_tensor("out", [T, H], F32, kind="ExternalOutput").ap()

    n_ht = H // P   # 16 contraction tiles of matmul 1
    n_tb = T // TB  # 2  token free-dim blocks in matmul 1
    n_ft = F // P   # 32 f-tiles (contraction tiles of matmul 2)
    n_tt = T // P   # 8  token psum tiles of matmul 2
    n_hb = H // TB  # 4  512-wide h blocks of W2
    C_TAIL = 4      # f-steps of matmul 2 issued per-tt to stagger drains

    with ExitStack() as ctx:
        hstage = ctx.enter_context(tc.tile_pool(name="hstage", bufs=2))
        wstage = ctx.enter_context(tc.tile_pool(name="wstage", bufs=8))
        htp = ctx.enter_context(tc.tile_pool(name="htp", bufs=n_ht))
        actp = ctx.enter_context(tc.tile_pool(name="actp", bufs=n_ft))
        w1bp = ctx.enter_context(tc.tile_pool(name="w1bp", bufs=48))
        w2bp = ctx.enter_context(tc.tile_pool(name="w2bp", bufs=22))
        silp = ctx.enter_context(tc.tile_pool(name="silp", bufs=4))
        outp = ctx.enter_context(tc.tile_pool(name="outp", bufs=3))
        ps = ctx.enter_context(tc.tile_pool(name="ps", bufs=8, space="PSUM"))

        ht = [htp.tile([P, T], BF16, tag="ht", name=f"ht{i}") for i in range(n_ht)]
        act = [actp.tile([P, T], BF16, tag="act", name=f"act{i}") for i in range(n_ft)]

        def load_h(hh, split=False):
            st = hstage.tile([P, T], F32, tag="hst", name=f"hst{hh}")
            if split:  # first stripes: halve DMA+cast so matmul 0 starts sooner
                for q in range(2):
                    sl = slice(q * TB, (q + 1) * TB)
                    nc.sync.dma_start(st[:, sl], h_d[hh * P : (hh + 1) * P, sl])
                    nc.vector.tensor_copy(out=ht[hh][:, sl], in_=st[:, sl])
            else:
                nc.sync.dma_start(st[:], h_d[hh * P : (hh + 1) * P, :])
                nc.vector.tensor_copy(out=ht[hh][:], in_=st[:])

        w1t = {}  # block -> {x -> [bf16 tile per hh]}

        def load_w1(b, x, hh):
            """DMA + cast one [128, width] stripe of W1 half x (0=gate, 1=up)."""
            f0, w = BLOCKS[b]
            c0 = x * F + f0
            st = wstage.tile([P, w], F32, tag="wst", name=f"w1s_{b}_{x}_{hh}")
            nc.sync.dma_start(st[:], w1_d[hh * P : (hh + 1) * P, c0 : c0 + w])
            wb = w1bp.tile([P, w], BF16, tag="w1b", name=f"w1b_{b}_{x}_{hh}")
            nc.vector.tensor_copy(out=wb[:], in_=st[:])
            w1t.setdefault(b, {0: [None] * n_ht, 1: [None] * n_ht})[x][hh] = wb

        w2t = {}  # (hb, f) -> bf16 tile

        def load_w2(hb, f):
            """DMA + cast one [128, 512] tile of W2 (f-tile f, h-block hb)."""
            st = wstage.tile([P, TB], F32, tag="wst", name=f"w2s_{hb}_{f}")
            nc.sync.dma_start(
                st[:], w2_d[f * P : (f + 1) * P, hb * TB : (hb + 1) * TB]
            )
            wb = w2bp.tile([P, TB], BF16, tag="w2b", name=f"w2b_{hb}_{f}")
            nc.vector.tensor_copy(out=wb[:], in_=st[:])
            w2t[(hb, f)] = wb

        def swiglu(fi, pg, pu):
            for tb in range(n_tb):
                sig = silp.tile([P, TB], BF16, tag="silp", name=f"sig{fi}_{tb}")
                nc.scalar.activation(sig[:], pg[tb][:], ACT_SIGMOID)
                tmp = silp.tile([P, TB], BF16, tag="tmpp", name=f"tmp{fi}_{tb}")
                nc.vector.tensor_mul(out=tmp[:], in0=pu[tb][:], in1=sig[:])
                nc.vector.tensor_mul(
                    out=act[fi][:, tb * TB : (tb + 1) * TB],
                    in0=tmp[:],
                    in1=pg[tb][:],
                )

        # ---- PE warm-up: a short burst of throwaway matmuls on scratch data
        # trips the HAM activity window (~3.4 us) while the first input
        # stripes stream in, so real matmuls start at 2.4 GHz instead of 1.2.
        warm = silp.tile([P, TB], BF16, tag="silp", name="warm")
        nc.vector.memset(warm[:], 0)
        # 8 scratch PSUM tiles keep the ps-pool slot rotation 8-aligned so
        # later allocations land on the same banks as without the warm-up.
        for b in range(8):
            pwarm = ps.tile([P, TB], F32, tag="ps", name=f"pwarm{b}")
            for k in range(3):
                nc.tensor.matmul(
                    pwarm[:],
                    lhsT=warm[:, :P],
                    rhs=warm[:],
                    start=(k == 0),
                    stop=(k == 2),
                )

        # ---- Startup: interleave h stripes with the narrow first W1 block
        # per contraction tile so matmul work unlocks every ~0.75 MB.
        for hh in range(n_ht):
            load_h(hh, split=(hh < 2))
            for x in range(2):
                load_w1(0, x, hh)

        def mm1_block(b, prefetch):
            """Matmul-1 + SwiGLU for one W1 f-block (gate+up halves).

            prefetch: list of thunks; one is popped and run after each
            i-phase's matmuls to place next-block DMAs in program order.
            """
            f0, w = BLOCKS[b]
            wg, wu = w1t[b][0], w1t[b][1]
            prev = None
            for i in range(w // P):
                fi = f0 // P + i
                pg = [
                    ps.tile([P, TB], F32, tag="ps", name=f"pg{fi}_{tb}")
                    for tb in range(n_tb)
                ]
                pu = [
                    ps.tile([P, TB], F32, tag="ps", name=f"pu{fi}_{tb}")
                    for tb in range(n_tb)
                ]
                for hh in range(n_ht):
                    first, last = hh == 0, hh == n_ht - 1
                    for p, wt in ((pg, wg), (pu, wu)):
                        lw = wt[hh][:, i * P : (i + 1) * P]
                        for tb in range(n_tb):
                            nc.tensor.matmul(
                                p[tb][:],
                                lhsT=lw,
                                rhs=ht[hh][:, tb * TB : (tb + 1) * TB],
                                start=first,
                                stop=last,
                            )
                if prev is not None:
                    swiglu(*prev)
                if prefetch:
                    prefetch.pop(0)()
                prev = (fi, pg, pu)
            swiglu(*prev)

        def w1_prefetch(b_next, nphases):
            """Thunks loading block b_next's 32 stripes across nphases."""
            stripes = [(x, hh) for hh in range(n_ht) for x in range(2)]
            per = len(stripes) // nphases

            def mk(chunk):
                def go():
                    for x, hh in chunk:
                        load_w1(b_next, x, hh)

                return go

            return [
                mk(stripes[k * per : (k + 1) * per]) for k in range(nphases)
            ]

        w2q = [(0, f) for f in range(n_ft)]  # h-block 0 tiles to prefetch

        def with_w2(pf):
            def wrap(thunk):
                def go():
                    thunk()
                    for _ in range(4):
                        if w2q:
                            load_w2(*w2q.pop(0))

                return go

            return [wrap(t) for t in pf]

        n_blk = len(BLOCKS)
        for b in range(n_blk):
            nif = BLOCKS[b][1] // P
            if b < n_blk - 1:
                pf = w1_prefetch(b + 1, nif)
            else:
                pf = [lambda: None for _ in range(nif)]
            if b >= n_blk - 3:
                pf = with_w2(pf)
            mm1_block(b, pf)

        # ---- Phase C: out = act @ W2, f contraction innermost across all 8
        # token-tile accumulators; W2 tiles stream one-hb-ahead through the
        # rotating pool.
        for hb in range(n_hb):
            po = [
                ps.tile([P, TB], F32, tag="ps", name=f"po{hb}_{tt}")
                for tt in range(n_tt)
            ]
            for f in range(n_ft):
                if hb + 1 < n_hb:
                    load_w2(hb + 1, f)
                if f < n_ft - C_TAIL:
                    wt = w2t.pop((hb, f))
                    for tt in range(n_tt):
                        nc.tensor.matmul(
                            po[tt][:],
                            lhsT=act[f][:, tt * P : (tt + 1) * P],
                            rhs=wt[:],
                            start=(f == 0),
                            stop=False,
                        )
            # staggered tail: per token tile, finish the last f-steps, then
            # drain + store while the next token tile computes
            for tt in range(n_tt):
                for f in range(n_ft - C_TAIL, n_ft):
                    nc.tensor.matmul(
                        po[tt][:],
                        lhsT=act[f][:, tt * P : (tt + 1) * P],
                        rhs=w2t[(hb, f)][:],
                        start=False,
                        stop=(f == n_ft - 1),
                    )
                ob = outp.tile([P, TB], F32, tag="outp", name=f"ob{hb}_{tt}")
                if hb == n_hb - 1 and tt == n_tt - 1:
                    # final tile: drain in halves to shorten the serial tail
                    for q in range(2):
                        sl = slice(q * (TB // 2), (q + 1) * (TB // 2))
                        nc.vector.tensor_copy(out=ob[:, sl], in_=po[tt][:, sl])
                        nc.sync.dma_start(
                            out_d[
                                tt * P : (tt + 1) * P,
                                hb * TB + q * (TB // 2) : hb * TB
                                + (q + 1) * (TB // 2),
                            ],
                            ob[:, sl],
                        )
                else:
                    nc.vector.tensor_copy(out=ob[:], in_=po[tt][:])
                    nc.sync.dma_start(
                        out_d[tt * P : (tt + 1) * P, hb * TB : (hb + 1) * TB],
                        ob[:],
                    )
            for f in range(n_ft - C_TAIL, n_ft):
                del w2t[(hb, f)]


def build_nc(T=T, H=H, F=F):
    nc = bacc.Bacc(
        "TRN2", target_bir_lowering=False, debug=False, enable_asserts=False
    )
    with tile.TileContext(nc) as tc:
        build_kernel_body(tc, T=T, H=H, F=F)
    nc.compile()
    return nc


_NC_CACHE = None


def run(hidden_states, gate_up_proj, down_proj, trace=False, **kw):
    """Run on the 8 NeuronCores; returns (output, BassKernelResults)."""
    global _NC_CACHE
    if _NC_CACHE is None:
        _NC_CACHE = build_nc()
    nc = _NC_CACHE

    hs = np.ascontiguousarray(np.asarray(hidden_states), dtype=np.float32)
    gup = np.ascontiguousarray(np.asarray(gate_up_proj), dtype=np.float32)
    dp = np.ascontiguousarray(np.asarray(down_proj), dtype=np.float32)
    assert hs.shape == (N_CORES * T, H), hs.shape
    assert gup.shape == (N_CORES, H, 2 * F), gup.shape
    assert dp.shape == (N_CORES, F, H), dp.shape

    in_maps = [
        {
            "hidden_states": np.ascontiguousarray(hs[i * T : (i + 1) * T].T),
            "gate_up_proj": np.ascontiguousarray(gup[i]),
            "down_proj": np.ascontiguousarray(dp[i]),
        }
        for i in range(N_CORES)
    ]
    res = run_bass_kernel_spmd(
        nc, in_maps, core_ids=list(range(N_CORES)), trace=trace, **kw
    )
    out = np.concatenate(
        [res.results[i]["out"] for i in range(N_CORES)], axis=0
    ).astype(np.float32)
    return out, res


def kernel(hidden_states, gate_up_proj, down_proj):
    out, _ = run(hidden_states, gate_up_proj, down_proj, trace=False)
    return out


# revision 19
# speedup vs baseline: 1.0145x; 1.0145x over previous
"""Llama4-style MoE experts (grouped SwiGLU MLP) on Trainium2, 8 NeuronCores.

Expert-parallel: core i runs expert i's full MLP on its 1024-token slice:
    out = (up * silu(gate)) @ W2,  [gate|up] = h @ W1
Per-core shapes: hT [2048, 1024] (host pre-transposed), W1 [2048, 8192],
W2 [4096, 2048].

hidden_states is transposed on the host so h arrives contraction-major;
no on-chip transposes. All matmuls run bf16 on the TensorEngine (f32
operands are cast on the otherwise-idle VectorEngine). Phase B (h @ W1 +
SwiGLU) walks W1 f-blocks [256, 512*7, 256]: the narrow first block
keeps the DMA-bound startup working set small (h + 4 MB) and its
stripes interleave with the h stream per contraction tile, so matmuls
unlock every ~0.75 MB delivered. Phase C (act @ W2) keeps all 8
token-tile PSUM accumulators live and walks the f contraction
innermost, so each W2 tile is consumed in one burst right after it
lands and a small rotating pool gives a deep prefetch; the last 4
f-steps are issued per-token-tile so output drains stagger instead of
bunching at the end.
"""

from contextlib import ExitStack

import numpy as np

import concourse.bass as bass
import concourse.mybir as mybir
import concourse.tile as tile
from concourse import bacc
from concourse.bass_utils import run_bass_kernel_spmd

N_CORES = 8
P = 128
TB = 512  # PSUM free-dim block (one f32 bank)

F32 = mybir.dt.float32
BF16 = mybir.dt.bfloat16
ACT_SIGMOID = mybir.ActivationFunctionType.Sigmoid

# Per-core problem dims (full problem: 8 experts x 1024 tokens, H=2048, F=4096)
T = 1024
H = 2048
F = 4096

# W1 f-block widths (per gate/up half): narrow edges, wide middle
BLOCKS = [(0, 256)] + [(256 + 512 * k, 512) for k in range(7)] + [(3840, 256)]


def build_kernel_body(tc, T=T, H=H, F=F):
    nc = tc.nc
    h_d = nc.dram_tensor("hidden_states", [H, T], F32, kind="ExternalInput").ap()
    w1_d = nc.dram_tensor("gate_up_proj", [H, 2 * F], F32, kind="ExternalInput").ap()
    w2_d = nc.dram_tensor("down_proj", [F, H], F32, kind="ExternalInput").ap()
    # out is written transposed [H, T]; the host transposes it back, the
    # same way hidden_states arrives pre-transposed. This lets phase C keep
    # W2 stationary (one LDWEIGHTS per two matmuls) with act moving.
    out_d = nc.dram_tensor("out", [H, T], F32, kind="ExternalOutput").ap()

    n_ht = H // P   # 16 contraction tiles of matmul 1
    n_tb = T // TB  # 2  token free-dim blocks in matmul 1
    n_ft = F // P   # 32 f-tiles (contraction tiles of matmul 2)
    n_tt = T // P   # 8  token psum tiles of matmul 2
    n_hb = H // TB  # 4  512-wide h blocks of W2
    C_TAIL = 4      # f-steps of matmul 2 issued per-tt to stagger drains

    with ExitStack() as ctx:
        hstage = ctx.enter_context(tc.tile_pool(name="hstage", bufs=2))
        wstage = ctx.enter_context(tc.tile_pool(name="wstage", bufs=8))
        htp = ctx.enter_context(tc.tile_pool(name="htp", bufs=n_ht))
        actp = ctx.enter_context(tc.tile_pool(name="actp", bufs=n_ft))
        w1bp = ctx.enter_context(tc.tile_pool(name="w1bp", bufs=48))
        w2bp = ctx.enter_context(tc.tile_pool(name="w2bp", bufs=22))
        silp = ctx.enter_context(tc.tile_pool(name="silp", bufs=4))
        outp = ctx.enter_context(tc.tile_pool(name="outp", bufs=3))
        ps = ctx.enter_context(tc.tile_pool(name="ps", bufs=8, space="PSUM"))

        ht = [htp.tile([P, T], BF16, tag="ht", name=f"ht{i}") for i in range(n_ht)]
        act = [actp.tile([P, T], BF16, tag="act", name=f"act{i}") for i in range(n_ft)]

        def load_h(hh, split=False):
            st = hstage.tile([P, T], F32, tag="hst", name=f"hst{hh}")
            if split:  # first stripes: halve DMA+cast so matmul 0 starts sooner
                for q in range(2):
                    sl = slice(q * TB, (q + 1) * TB)
                    nc.sync.dma_start(st[:, sl], h_d[hh * P : (hh + 1) * P, sl])
                    nc.vector.tensor_copy(out=ht[hh][:, sl], in_=st[:, sl])
            else:
                nc.sync.dma_start(st[:], h_d[hh * P : (hh + 1) * P, :])
                nc.vector.tensor_copy(out=ht[hh][:], in_=st[:])

        w1t = {}  # block -> {x -> [bf16 tile per hh]}

        def load_w1(b, x, hh):
            """DMA + cast one [128, width] stripe of W1 half x (0=gate, 1=up)."""
            f0, w = BLOCKS[b]
            c0 = x * F + f0
            st = wstage.tile([P, w], F32, tag="wst", name=f"w1s_{b}_{x}_{hh}")
            nc.sync.dma_start(st[:], w1_d[hh * P : (hh + 1) * P, c0 : c0 + w])
            wb = w1bp.tile([P, w], BF16, tag="w1b", name=f"w1b_{b}_{x}_{hh}")
            nc.vector.tensor_copy(out=wb[:], in_=st[:])
            w1t.setdefault(b, {0: [None] * n_ht, 1: [None] * n_ht})[x][hh] = wb

        w2t = {}  # (hb, f) -> bf16 tile

        def load_w2(hb, f):
            """DMA + cast one [128, 512] tile of W2 (f-tile f, h-block hb)."""
            st = wstage.tile([P, TB], F32, tag="wst", name=f"w2s_{hb}_{f}")
            nc.sync.dma_start(
                st[:], w2_d[f * P : (f + 1) * P, hb * TB : (hb + 1) * TB]
            )
            wb = w2bp.tile([P, TB], BF16, tag="w2b", name=f"w2b_{hb}_{f}")
            nc.vector.tensor_copy(out=wb[:], in_=st[:])
            w2t[(hb, f)] = wb

        def swiglu(fi, pg, pu):
            for tb in range(n_tb):
                sig = silp.tile([P, TB], BF16, tag="silp", name=f"sig{fi}_{tb}")
                nc.scalar.activation(sig[:], pg[tb][:], ACT_SIGMOID)
                tmp = silp.tile([P, TB], BF16, tag="tmpp", name=f"tmp{fi}_{tb}")
                nc.vector.tensor_mul(out=tmp[:], in0=pu[tb][:], in1=sig[:])
                nc.vector.tensor_mul(
                    out=act[fi][:, tb * TB : (tb + 1) * TB],
                    in0=tmp[:],
                    in1=pg[tb][:],
                )

        # ---- PE warm-up: a short burst of throwaway matmuls on scratch data
        # trips the HAM activity window (~3.4 us) while the first input
        # stripes stream in, so real matmuls start at 2.4 GHz instead of 1.2.
        warm = silp.tile([P, TB], BF16, tag="silp", name="warm")
        nc.vector.memset(warm[:], 0)
        # 8 scratch PSUM tiles keep the ps-pool slot rotation 8-aligned so
        # later allocations land on the same banks as without the warm-up.
        for b in range(8):
            pwarm = ps.tile([P, TB], F32, tag="ps", name=f"pwarm{b}")
            for k in range(3):
                nc.tensor.matmul(
                    pwarm[:],
                    lhsT=warm[:, :P],
                    rhs=warm[:],
                    start=(k == 0),
                    stop=(k == 2),
                )

        # ---- Startup: interleave h stripes with the narrow first W1 block
        # per contraction tile so matmul work unlocks every ~0.75 MB.
        for hh in range(n_ht):
            load_h(hh, split=(hh < 2))
            for x in range(2):
                load_w1(0, x, hh)

        def mm1_block(b, prefetch):
            """Matmul-1 + SwiGLU for one W1 f-block (gate+up halves).

            prefetch: list of thunks; one is popped and run after each
            i-phase's matmuls to place next-block DMAs in program order.
            """
            f0, w = BLOCKS[b]
            wg, wu = w1t[b][0], w1t[b][1]
            prev = None
            for i in range(w // P):
                fi = f0 // P + i
                pg = [
                    ps.tile([P, TB], F32, tag="ps", name=f"pg{fi}_{tb}")
                    for tb in range(n_tb)
                ]
                pu = [
                    ps.tile([P, TB], F32, tag="ps", name=f"pu{fi}_{tb}")
                    for tb in range(n_tb)
                ]
                for hh in range(n_ht):
                    first, last = hh == 0, hh == n_ht - 1
                    for p, wt in ((pg, wg), (pu, wu)):
                        lw = wt[hh][:, i * P : (i + 1) * P]
                        for tb in range(n_tb):
                            nc.tensor.matmul(
                                p[tb][:],
                                lhsT=lw,
                                rhs=ht[hh][:, tb * TB : (tb + 1) * TB],
                                start=first,
                                stop=last,
                            )
                if prev is not None:
                    swiglu(*prev)
                if prefetch:
                    prefetch.pop(0)()
                prev = (fi, pg, pu)
            swiglu(*prev)

        def w1_prefetch(b_next, nphases):
            """Thunks loading block b_next's 32 stripes across nphases."""
            stripes = [(x, hh) for hh in range(n_ht) for x in range(2)]
            per = len(stripes) // nphases

            def mk(chunk):
                def go():
                    for x, hh in chunk:
                        load_w1(b_next, x, hh)

                return go

            return [
                mk(stripes[k * per : (k + 1) * per]) for k in range(nphases)
            ]

        w2q = [(0, f) for f in range(n_ft)]  # h-block 0 tiles to prefetch

        def with_w2(pf):
            def wrap(thunk):
                def go():
                    thunk()
                    for _ in range(4):
                        if w2q:
                            load_w2(*w2q.pop(0))

                return go

            return [wrap(t) for t in pf]

        n_blk = len(BLOCKS)
        for b in range(n_blk):
            nif = BLOCKS[b][1] // P
            if b < n_blk - 1:
                pf = w1_prefetch(b + 1, nif)
            else:
                pf = [lambda: None for _ in range(nif)]
            if b >= n_blk - 3:
                pf = with_w2(pf)
            mm1_block(b, pf)

        # ---- Phase C: outT = (act @ W2)T, f contraction innermost across 8
        # live accumulators (4 h-columns x 2 token halves per h-block pass).
        # W2 is the stationary operand — one LDWEIGHTS serves both token
        # halves — and each W2 tile streams through in one 8-matmul burst.
        for hb in range(n_hb):
            po = [
                ps.tile([P, TB], F32, tag="ps", name=f"po{hb}_{k}")
                for k in range(8)
            ]
            for f in range(n_ft):
                if hb + 1 < n_hb:
                    load_w2(hb + 1, f)
                if f < n_ft - C_TAIL:
                    wt = w2t.pop((hb, f))
                    for hc in range(4):
                        lw = wt[:, hc * P : (hc + 1) * P]
                        for th in range(n_tb):
                            nc.tensor.matmul(
                                po[hc * n_tb + th][:],
                                lhsT=lw,
                                rhs=act[f][:, th * TB : (th + 1) * TB],
                                start=(f == 0),
                                stop=False,
                            )
            # staggered tail: per accumulator, finish the last f-steps, then
            # drain + store while the next accumulator computes
            for k in range(8):
                hc, th = divmod(k, n_tb)
                for f in range(n_ft - C_TAIL, n_ft):
                    nc.tensor.matmul(
                        po[k][:],
                        lhsT=w2t[(hb, f)][:, hc * P : (hc + 1) * P],
                        rhs=act[f][:, th * TB : (th + 1) * TB],
                        start=False,
                        stop=(f == n_ft - 1),
                    )
                ob = outp.tile([P, TB], F32, tag="outp", name=f"ob{hb}_{k}")
                r0 = hb * TB + hc * P
                if hb == n_hb - 1 and k == 7:
                    # final tile: drain in halves to shorten the serial tail
                    for q in range(2):
                        sl = slice(q * (TB // 2), (q + 1) * (TB // 2))
                        nc.vector.tensor_copy(out=ob[:, sl], in_=po[k][:, sl])
                        nc.sync.dma_start(
                            out_d[
                                r0 : r0 + P,
                                th * TB + q * (TB // 2) : th * TB
                                + (q + 1) * (TB // 2),
                            ],
                            ob[:, sl],
                        )
                else:
                    nc.vector.tensor_copy(out=ob[:], in_=po[k][:])
                    nc.sync.dma_start(
                        out_d[r0 : r0 + P, th * TB : (th + 1) * TB], ob[:]
                    )
            for f in range(n_ft - C_TAIL, n_ft):
                del w2t[(hb, f)]


def build_nc(T=T, H=H, F=F):
    nc = bacc.Bacc(
        "TRN2", target_bir_lowering=False, debug=False, enable_asserts=False
    )
    with tile.TileContext(nc) as tc:
        build_kernel_body(tc, T=T, H=H, F=F)
    nc.compile()
    return nc


_NC_CACHE = None


def run(hidden_states, gate_up_proj, down_proj, trace=False, **kw):
    """Run on the 8 NeuronCores; returns (output, BassKernelResults)."""
    global _NC_CACHE
    if _NC_CACHE is None:
        _NC_CACHE = build_nc()
    nc = _NC_CACHE

    hs = np.ascontiguousarray(np.asarray(hidden_states), dtype=np.float32)
    gup = np.ascontiguousarray(np.asarray(gate_up_proj), dtype=np.float32)
    dp = np.ascontiguousarray(np.asarray(down_proj), dtype=np.float32)
    assert hs.shape == (N_CORES * T, H), hs.shape
    assert gup.shape == (N_CORES, H, 2 * F), gup.shape
    assert dp.shape == (N_CORES, F, H), dp.shape

    in_maps = [
        {
            "hidden_states": np.ascontiguousarray(hs[i * T : (i + 1) * T].T),
            "gate_up_proj": np.ascontiguousarray(gup[i]),
            "down_proj": np.ascontiguousarray(dp[i]),
        }
        for i in range(N_CORES)
    ]
    res = run_bass_kernel_spmd(
        nc, in_maps, core_ids=list(range(N_CORES)), trace=trace, **kw
    )
    out = np.concatenate(
        [np.ascontiguousarray(res.results[i]["out"].T) for i in range(N_CORES)],
        axis=0,
    ).astype(np.float32)
    return out, res


def kernel(hidden_states, gate_up_proj, down_proj):
    out, _ = run(hidden_states, gate_up_proj, down_proj, trace=False)
    return out
